# Initial kernel scaffold
#
"""Trainium2 Bass kernel for DriverNet: 2-layer LSTM cell (single step, zero
initial state) + linear head over B=1M independent rows, data-parallel on 8
NeuronCores.

Math (per row, h0=c0=0, PyTorch gate order [i,f,g,o]; f-gate unused):
  g0 = x @ W_ih0.T + (b_ih0+b_hh0);  c1 = sig(g0_i)*tanh(g0_g);  h1 = sig(g0_o)*tanh(c1)
  g1 = h1 @ W_ih1.T + (b_ih1+b_hh1); c2 = sig(g1_i)*tanh(g1_g);  h2 = sig(g1_o)*tanh(c2)
  y  = tanh(h2 @ W_lin.T + b_lin)

Layout strategy (per core, 131072 rows = 1024 blocks of 128):
  - batch on SBUF partitions for all elementwise work
  - x is augmented host-side with a ones column (bias via matmul row)
  - PE transposes [128, chunk*22] -> [chunk*22, 128] give feature-major lhsT
  - block-diagonal weights (built host-side) evaluate `chunk` row-blocks per
    matmul: L0 chunk=4 blocks (K=88,N=60), L1 chunk=16 blocks (K=96,N=240)
  - gate columns grouped [i|o|g] so one strided Sigmoid covers i,o and one
    Tanh covers g, reading PSUM directly (evacuation == activation)
  - final linear = elementwise mul + tensor_reduce(X), tanh bias via ACT
"""

import os
import numpy as np
import ml_dtypes

B = 1 << 20
IN_DIM, HID, OUT_DIM = 21, 5, 1
NCORES = 8
BC = B // NCORES          # 131072 rows per core
NBLK = BC // 128          # 1024 blocks per core
SUPERS = [96] * 10 + [64]  # blocks per supertile
L0C = 4                   # L0 blocks per chunk  (K=88, N=60)
L1C = 16                  # L1 blocks per chunk  (K=96, N=240)
L0_PER_BANK = 8           # 8*60=480 <= 512 fp32
L1_PER_BANK = 2           # 2*240=480 <= 512

_CACHE = {}

# last BassKernelResults (for test harness timing introspection)
LAST_RESULTS = None


def _build_program():
    import concourse.bacc as bacc
    import concourse.tile as tile
    import concourse.mybir as mybir

    AF = mybir.ActivationFunctionType
    BF16 = mybir.dt.bfloat16
    F32 = mybir.dt.float32
    nc = bacc.Bacc("TRN2", target_bir_lowering=False, debug=False, num_devices=NCORES)

    x_d = nc.declare_dram_parameter("x22", [BC, 22], F32, isOutput=False)
    w0_d = nc.declare_dram_parameter("w0blk", [L0C * 22, L0C * 15], BF16, isOutput=False)
    w1_d = nc.declare_dram_parameter("w1blk", [L1C * 6, L1C * 15], BF16, isOutput=False)
    wr_d = nc.declare_dram_parameter("wrep", [128, 96 * HID], BF16, isOutput=False)
    bl_d = nc.declare_dram_parameter("blin", [128, 1], F32, isOutput=False)
    id_d = nc.declare_dram_parameter("ident", [128, 128], BF16, isOutput=False)
    y_d = nc.declare_dram_parameter("y", [BC, 1], F32, isOutput=True)

    with tile.TileContext(nc) as tc:
        with (
            tc.tile_pool(name="const", bufs=1) as constp,
            tc.tile_pool(name="xin", bufs=2) as xinp,
            tc.tile_pool(name="xt_ps", bufs=1, space="PSUM") as xtpsp,
            tc.tile_pool(name="xt_sb", bufs=2) as xtsbp,
            tc.tile_pool(name="g0_ps", bufs=1, space="PSUM") as g0psp,
            tc.tile_pool(name="h1t_ps", bufs=1, space="PSUM") as h1tpsp,
            tc.tile_pool(name="h1t_sb", bufs=2) as h1tsbp,
            tc.tile_pool(name="g1_ps", bufs=1, space="PSUM") as g1psp,
            tc.tile_pool(name="acts", bufs=2) as actsp,
            tc.tile_pool(name="yout", bufs=2) as youtp,
        ):
            w0_sb = constp.tile([L0C * 22, L0C * 15], BF16)
            nc.sync.dma_start(w0_sb[:], w0_d[:])
            w1_sb = constp.tile([L1C * 6, L1C * 15], BF16)
            nc.sync.dma_start(w1_sb[:], w1_d[:])
            wr_sb = constp.tile([128, 96 * HID], BF16)
            nc.sync.dma_start(wr_sb[:], wr_d[:])
            bl_sb = constp.tile([128, 1], F32)
            nc.sync.dma_start(bl_sb[:], bl_d[:])
            id_sb = constp.tile([128, 128], BF16)
            nc.sync.dma_start(id_sb[:], id_d[:])

            s0 = 0
            for nb in SUPERS:
                S = nb * 128
                n0ch = nb // L0C                  # 24 or 16
                n1ch = nb // L1C                  # 6 or 4
                g0b = n0ch // L0_PER_BANK         # 3 or 2 banks
                g1b = (n1ch + L1_PER_BANK - 1) // L1_PER_BANK

                # ---- load x (fp32 -> bf16 cast DMA, contiguous)
                x_tile = xinp.tile([128, nb * 22], BF16, tag="xin")
                nc.gpsimd.dma_start(
                    out=x_tile[:],
                    in_=x_d[s0 : s0 + S, :].rearrange("(p r) f -> p (r f)", p=128),
                )

                # ---- L0: transpose + matmul into g0 (3 banks)
                g0_ps = g0psp.tile([128, g0b * 512], F32, tag="g0")
                for bl in range(g0b):
                    ch_lo = bl * L0_PER_BANK
                    ch_hi = min(ch_lo + L0_PER_BANK, n0ch)
                    nch = ch_hi - ch_lo
                    xt_ps = xtpsp.tile([L0C * 22, L0_PER_BANK * 128], BF16, tag="xtps")
                    for c in range(nch):
                        nc.tensor.transpose(
                            xt_ps[:, c * 128 : (c + 1) * 128],
                            x_tile[:, (ch_lo + c) * L0C * 22 : (ch_lo + c + 1) * L0C * 22],
                            id_sb[:],
                        )
                    xt_sb = xtsbp.tile([L0C * 22, L0_PER_BANK * 128], BF16, tag="xtsb")
                    nc.vector.tensor_copy(xt_sb[:, : nch * 128], xt_ps[:, : nch * 128])
                    for c in range(nch):
                        nc.tensor.matmul(
                            g0_ps[:, bl * 512 + c * 60 : bl * 512 + (c + 1) * 60],
                            xt_sb[:, c * 128 : (c + 1) * 128],
                            w0_sb[:],
                            start=True,
                            stop=True,
                        )

                # ---- L0 activations (strided PSUM reads)
                g0v = (
                    g0_ps[:]
                    .rearrange("p (b x) -> p b x", x=512)[:, :, : L0_PER_BANK * 60]
                    .rearrange("p b (c n) -> p b c n", n=60)
                )
                sio0 = actsp.tile([128, n0ch * 40], BF16, tag="sio0")
                sio0v = sio0[:].rearrange("p (b c n) -> p b c n", n=40, c=L0_PER_BANK)
                nc.scalar.activation(sio0v, g0v[:, :, :, 0:40], AF.Sigmoid)
                tg0 = actsp.tile([128, n0ch * 20], BF16, tag="tg0")
                tg0v = tg0[:].rearrange("p (b c n) -> p b c n", n=20, c=L0_PER_BANK)
                nc.scalar.activation(tg0v, g0v[:, :, :, 40:60], AF.Tanh)

                c1 = actsp.tile([128, nb * HID], BF16, tag="c1")
                nc.vector.tensor_mul(
                    c1[:].rearrange("p (c n) -> p c n", n=20),
                    sio0[:].rearrange("p (c n) -> p c n", n=40)[:, :, 0:20],
                    tg0[:].rearrange("p (c n) -> p c n", n=20),
                )
                tc1 = actsp.tile([128, nb * HID], BF16, tag="tc1")
                nc.scalar.activation(tc1[:], c1[:], AF.Tanh)
                # h1 tile [128, nb, 6] with ones in slot 5 (bias row for L1)
                h1 = actsp.tile([128, nb * 6], BF16, tag="h1")
                h1v = h1[:].rearrange("p (r f) -> p r f", f=6)
                nc.vector.memset(h1v[:, :, 5:6], 1.0)
                nc.vector.tensor_mul(
                    h1v[:, :, 0:5].rearrange("p (c d) n -> p c (d n)", d=L0C),
                    sio0[:].rearrange("p (c n) -> p c n", n=40)[:, :, 20:40],
                    tc1[:].rearrange("p (c n) -> p c n", n=20),
                )

                # ---- L1: transpose + matmul into g1
                g1_ps = g1psp.tile([128, g1b * 512], F32, tag="g1")
                h1t_ps = h1tpsp.tile([L1C * 6, n1ch * 128], BF16, tag="h1tps")
                for c in range(n1ch):
                    nc.tensor.transpose(
                        h1t_ps[:, c * 128 : (c + 1) * 128],
                        h1[:, c * L1C * 6 : (c + 1) * L1C * 6],
                        id_sb[:],
                    )
                h1t_sb = h1tsbp.tile([L1C * 6, n1ch * 128], BF16, tag="h1tsb")
                nc.vector.tensor_copy(h1t_sb[:], h1t_ps[:])
                for c in range(n1ch):
                    off = (c // L1_PER_BANK) * 512 + (c % L1_PER_BANK) * 240
                    nc.tensor.matmul(
                        g1_ps[:, off : off + 240],
                        h1t_sb[:, c * 128 : (c + 1) * 128],
                        w1_sb[:],
                        start=True,
                        stop=True,
                    )

                # ---- L1 activations
                g1v = (
                    g1_ps[:]
                    .rearrange("p (b x) -> p b x", x=512)[:, :, : L1_PER_BANK * 240]
                    .rearrange("p b (c n) -> p b c n", n=240)
                )
                sio1 = actsp.tile([128, n1ch * 160], BF16, tag="sio1")
                sio1v = sio1[:].rearrange("p (b c n) -> p b c n", n=160, c=L1_PER_BANK)
                nc.scalar.activation(sio1v, g1v[:, :, :, 0:160], AF.Sigmoid)
                tg1 = actsp.tile([128, n1ch * 80], BF16, tag="tg1")
                tg1v = tg1[:].rearrange("p (b c n) -> p b c n", n=80, c=L1_PER_BANK)
                nc.scalar.activation(tg1v, g1v[:, :, :, 160:240], AF.Tanh)

                c2 = actsp.tile([128, nb * HID], BF16, tag="c2")
                nc.vector.tensor_mul(
                    c2[:].rearrange("p (c n) -> p c n", n=80),
                    sio1[:].rearrange("p (c n) -> p c n", n=160)[:, :, 0:80],
                    tg1[:].rearrange("p (c n) -> p c n", n=80),
                )
                tc2 = actsp.tile([128, nb * HID], BF16, tag="tc2")
                nc.scalar.activation(tc2[:], c2[:], AF.Tanh)
                v = actsp.tile([128, nb * HID], BF16, tag="v")
                nc.vector.tensor_mul(
                    v[:].rearrange("p (c n) -> p c n", n=80),
                    sio1[:].rearrange("p (c n) -> p c n", n=160)[:, :, 80:160],
                    tc2[:].rearrange("p (c n) -> p c n", n=80),
                )
                t = actsp.tile([128, nb * HID], BF16, tag="t")
                nc.vector.tensor_mul(t[:], v[:], wr_sb[:, : nb * HID])

                # ---- final reduce + tanh(+bias) + store
                ypre = actsp.tile([128, nb], F32, tag="ypre")
                nc.vector.tensor_reduce(
                    ypre[:].rearrange("p (r o) -> p r o", o=1),
                    t[:].rearrange("p (r f) -> p r f", f=HID),
                    mybir.AxisListType.X,
                    mybir.AluOpType.add,
                )
                y_tile = youtp.tile([128, nb], F32, tag="y")
                nc.scalar.activation(y_tile[:], ypre[:], AF.Tanh, bias=bl_sb[:, 0:1])
                nc.sync.dma_start(
                    out=y_d[s0 : s0 + S, 0].rearrange("(p r) -> p r", p=128),
                    in_=y_tile[:],
                )

                s0 += S

    nc.compile()
    return nc


def _build_inputs(x, W_ih0, W_hh0, b_ih0, b_hh0, W_ih1, W_hh1, b_ih1, b_hh1, W_lin, b_lin):
    bf16 = ml_dtypes.bfloat16
    b0 = (np.asarray(b_ih0) + np.asarray(b_hh0)).astype(np.float32)
    b1 = (np.asarray(b_ih1) + np.asarray(b_hh1)).astype(np.float32)
    W0 = np.asarray(W_ih0, np.float32)
    W1 = np.asarray(W_ih1, np.float32)
    sel = {"i": range(0, 5), "g": range(10, 15), "o": range(15, 20)}

    def blockdiag(W, b, chunk, krows):
        # krows per block = W.shape[1] (+1 bias row); col layout [i | o | g]
        kin = W.shape[1]
        out = np.zeros((chunk * (kin + 1), chunk * 15), np.float32)
        for dr in range(chunk):
            for grp, key in enumerate(("i", "o", "g")):
                for kk, gr in enumerate(sel[key]):
                    col = grp * (chunk * 5) + dr * 5 + kk
                    out[dr * (kin + 1) : dr * (kin + 1) + kin, col] = W[gr, :]
                    out[dr * (kin + 1) + kin, col] = b[gr]
        return out.astype(bf16)

    w0blk = blockdiag(W0, b0, L0C, 22)
    w1blk = blockdiag(W1, b1, L1C, 6)
    wrep = np.tile(np.asarray(W_lin, np.float32)[0], 96 * 128).reshape(128, 96 * HID).astype(bf16)
    blin = np.full((128, 1), float(np.asarray(b_lin)[0]), np.float32)
    ident = np.eye(128, dtype=bf16)

    x = np.asarray(x, np.float32)
    x22 = np.empty((B, 22), np.float32)
    x22[:, :21] = x
    x22[:, 21] = 1.0

    in_maps = []
    for c in range(NCORES):
        in_maps.append(
            {
                "x22": x22[c * BC : (c + 1) * BC],
                "w0blk": w0blk,
                "w1blk": w1blk,
                "wrep": wrep,
                "blin": blin,
                "ident": ident,
            }
        )
    return in_maps


def _reference_numpy(x, h0, c0, W_ih0, W_hh0, b_ih0, b_hh0, W_ih1, W_hh1, b_ih1, b_hh1, W_lin, b_lin):
    # general fallback (never taken for the spec'd zero-state inputs)
    def cell(x_, h, c, Wi, Wh, bi, bh):
        g = x_ @ Wi.T + h @ Wh.T + (bi + bh)
        i, f, gg, o = np.split(g, 4, axis=-1)
        sig = lambda z: 1.0 / (1.0 + np.exp(-z))
        cn = sig(f) * c + sig(i) * np.tanh(gg)
        return sig(o) * np.tanh(cn), cn

    h1, _ = cell(x, h0[0], c0[0], W_ih0, W_hh0, b_ih0, b_hh0)
    h2, _ = cell(h1, h0[1], c0[1], W_ih1, W_hh1, b_ih1, b_hh1)
    return np.tanh(h2 @ W_lin.T + b_lin).astype(np.float32)


def kernel(x, h0, c0, W_ih0, W_hh0, b_ih0, b_hh0, W_ih1, W_hh1, b_ih1, b_hh1, W_lin, b_lin):
    global LAST_RESULTS
    args = dict(
        x=np.asarray(x), h0=np.asarray(h0), c0=np.asarray(c0),
        W_ih0=np.asarray(W_ih0), W_hh0=np.asarray(W_hh0),
        b_ih0=np.asarray(b_ih0), b_hh0=np.asarray(b_hh0),
        W_ih1=np.asarray(W_ih1), W_hh1=np.asarray(W_hh1),
        b_ih1=np.asarray(b_ih1), b_hh1=np.asarray(b_hh1),
        W_lin=np.asarray(W_lin), b_lin=np.asarray(b_lin),
    )
    if np.any(args["h0"]) or np.any(args["c0"]):
        return _reference_numpy(**args)

    from concourse.bass_utils import run_bass_kernel_spmd

    if "nc" not in _CACHE:
        _CACHE["nc"] = _build_program()
    nc = _CACHE["nc"]

    in_maps = _build_inputs(
        args["x"], args["W_ih0"], args["W_hh0"], args["b_ih0"], args["b_hh0"],
        args["W_ih1"], args["W_hh1"], args["b_ih1"], args["b_hh1"],
        args["W_lin"], args["b_lin"],
    )
    trace = bool(int(os.environ.get("TRN_TRACE", "0")))
    res = run_bass_kernel_spmd(nc, in_maps, list(range(NCORES)), trace=trace)
    LAST_RESULTS = res
    return np.concatenate([res.results[i]["y"] for i in range(NCORES)], axis=0)


# revision 7
# speedup vs baseline: 5.4868x; 5.4868x over previous
"""Trainium2 Bass kernel for DriverNet: 2-layer LSTM cell (single step, zero
initial state) + linear head over B=1M independent rows, data-parallel on 8
NeuronCores.

Math (per row, h0=c0=0, PyTorch gate order [i,f,g,o]; f-gate unused):
  g0 = x @ W_ih0.T + (b_ih0+b_hh0);  c1 = sig(g0_i)*tanh(g0_g);  h1 = sig(g0_o)*tanh(c1)
  g1 = h1 @ W_ih1.T + (b_ih1+b_hh1); c2 = sig(g1_i)*tanh(g1_g);  h2 = sig(g1_o)*tanh(c2)
  y  = tanh(h2 @ W_lin.T + b_lin)

Layout strategy (per core, 131072 rows = 1024 blocks of 128):
  - batch on SBUF partitions for all elementwise work
  - x is augmented host-side with a ones column (bias via matmul row)
  - PE transposes [128, chunk*22] -> [chunk*22, 128] give feature-major lhsT
  - block-diagonal weights (built host-side) evaluate `chunk` row-blocks per
    matmul: L0 chunk=4 blocks (K=88,N=60), L1 chunk=16 blocks (K=96,N=240)
  - gate columns grouped [i|o|g] so one strided Sigmoid covers i,o and one
    Tanh covers g, reading PSUM directly (evacuation == activation)
  - final linear = elementwise mul + tensor_reduce(X), tanh bias via ACT
"""

import os
import numpy as np
import ml_dtypes

B = 1 << 20
IN_DIM, HID, OUT_DIM = 21, 5, 1
NCORES = 8
BC = B // NCORES          # 131072 rows per core
NBLK = BC // 128          # 1024 blocks per core
SUPERS = [96] * 10 + [64]  # blocks per supertile
L0C = 4                   # L0 blocks per chunk  (K=88, N=60)
L1C = 16                  # L1 blocks per chunk  (K=96, N=240)
L0_PER_BANK = 8           # 8*60=480 <= 512 fp32
L1_PER_BANK = 2           # 2*240=480 <= 512

_CACHE = {}

# last BassKernelResults (for test harness timing introspection)
LAST_RESULTS = None


def _build_program(reps=1):
    import concourse.bacc as bacc
    import concourse.tile as tile
    import concourse.mybir as mybir

    AF = mybir.ActivationFunctionType
    BF16 = mybir.dt.bfloat16
    F32 = mybir.dt.float32
    nc = bacc.Bacc("TRN2", target_bir_lowering=False, debug=False, num_devices=NCORES)

    x_d = nc.declare_dram_parameter("x22", [BC, 22], F32, isOutput=False)
    w0_d = nc.declare_dram_parameter("w0blk", [L0C * 22, L0C * 15], BF16, isOutput=False)
    w1_d = nc.declare_dram_parameter("w1blk", [L1C * 6, L1C * 15], BF16, isOutput=False)
    wr_d = nc.declare_dram_parameter("wrep", [128, 96 * HID], BF16, isOutput=False)
    bl_d = nc.declare_dram_parameter("blin", [128, 1], F32, isOutput=False)
    id_d = nc.declare_dram_parameter("ident", [128, 128], BF16, isOutput=False)
    y_d = nc.declare_dram_parameter("y", [BC, 1], F32, isOutput=True)

    with tile.TileContext(nc) as tc:
        with (
            tc.tile_pool(name="const", bufs=1) as constp,
            tc.tile_pool(name="xin", bufs=2) as xinp,
            tc.tile_pool(name="xt_ps", bufs=1, space="PSUM") as xtpsp,
            tc.tile_pool(name="xt_sb", bufs=2) as xtsbp,
            tc.tile_pool(name="g0_ps", bufs=1, space="PSUM") as g0psp,
            tc.tile_pool(name="h1t_ps", bufs=1, space="PSUM") as h1tpsp,
            tc.tile_pool(name="h1t_sb", bufs=2) as h1tsbp,
            tc.tile_pool(name="g1_ps", bufs=1, space="PSUM") as g1psp,
            tc.tile_pool(name="acts", bufs=2) as actsp,
            tc.tile_pool(name="yout", bufs=2) as youtp,
        ):
            w0_sb = constp.tile([L0C * 22, L0C * 15], BF16)
            nc.sync.dma_start(w0_sb[:], w0_d[:])
            w1_sb = constp.tile([L1C * 6, L1C * 15], BF16)
            nc.sync.dma_start(w1_sb[:], w1_d[:])
            wr_sb = constp.tile([128, 96 * HID], BF16)
            nc.sync.dma_start(wr_sb[:], wr_d[:])
            bl_sb = constp.tile([128, 1], F32)
            nc.sync.dma_start(bl_sb[:], bl_d[:])
            id_sb = constp.tile([128, 128], BF16)
            nc.sync.dma_start(id_sb[:], id_d[:])

            import contextlib

            if reps > 1:
                rep_ctx = tc.For_i(0, reps, 1, hint_engines=tuple(nc.engines))
            else:
                rep_ctx = contextlib.nullcontext()
            with rep_ctx:
              s0 = 0
              for nb in SUPERS:
                S = nb * 128
                n0ch = nb // L0C                  # 24 or 16
                n1ch = nb // L1C                  # 6 or 4
                g0b = n0ch // L0_PER_BANK         # 3 or 2 banks
                g1b = (n1ch + L1_PER_BANK - 1) // L1_PER_BANK

                # ---- load x (fp32 -> bf16 cast DMA, contiguous)
                x_tile = xinp.tile([128, nb * 22], BF16, tag="xin")
                nc.gpsimd.dma_start(
                    out=x_tile[:],
                    in_=x_d[s0 : s0 + S, :].rearrange("(p r) f -> p (r f)", p=128),
                )

                # ---- L0: transpose + matmul into g0 (3 banks)
                g0_ps = g0psp.tile([128, g0b * 512], F32, tag="g0")
                for bl in range(g0b):
                    ch_lo = bl * L0_PER_BANK
                    ch_hi = min(ch_lo + L0_PER_BANK, n0ch)
                    nch = ch_hi - ch_lo
                    xt_ps = xtpsp.tile([L0C * 22, L0_PER_BANK * 128], BF16, tag="xtps")
                    for c in range(nch):
                        nc.tensor.transpose(
                            xt_ps[:, c * 128 : (c + 1) * 128],
                            x_tile[:, (ch_lo + c) * L0C * 22 : (ch_lo + c + 1) * L0C * 22],
                            id_sb[:],
                        )
                    xt_sb = xtsbp.tile([L0C * 22, L0_PER_BANK * 128], BF16, tag="xtsb")
                    nc.vector.tensor_copy(xt_sb[:, : nch * 128], xt_ps[:, : nch * 128])
                    for c in range(nch):
                        nc.tensor.matmul(
                            g0_ps[:, bl * 512 + c * 60 : bl * 512 + (c + 1) * 60],
                            xt_sb[:, c * 128 : (c + 1) * 128],
                            w0_sb[:],
                            start=True,
                            stop=True,
                        )

                # ---- L0 activations (strided PSUM reads)
                g0v = (
                    g0_ps[:]
                    .rearrange("p (b x) -> p b x", x=512)[:, :, : L0_PER_BANK * 60]
                    .rearrange("p b (c n) -> p b c n", n=60)
                )
                sio0 = actsp.tile([128, n0ch * 40], BF16, tag="sio0")
                sio0v = sio0[:].rearrange("p (b c n) -> p b c n", n=40, c=L0_PER_BANK)
                nc.scalar.activation(sio0v, g0v[:, :, :, 0:40], AF.Sigmoid)
                tg0 = actsp.tile([128, n0ch * 20], BF16, tag="tg0")
                tg0v = tg0[:].rearrange("p (b c n) -> p b c n", n=20, c=L0_PER_BANK)
                nc.scalar.activation(tg0v, g0v[:, :, :, 40:60], AF.Tanh)

                c1 = actsp.tile([128, nb * HID], BF16, tag="c1")
                nc.vector.tensor_mul(
                    c1[:].rearrange("p (c n) -> p c n", n=20),
                    sio0[:].rearrange("p (c n) -> p c n", n=40)[:, :, 0:20],
                    tg0[:].rearrange("p (c n) -> p c n", n=20),
                )
                tc1 = actsp.tile([128, nb * HID], BF16, tag="tc1")
                nc.scalar.activation(tc1[:], c1[:], AF.Tanh)
                # h1 tile [128, nb, 6] with ones in slot 5 (bias row for L1)
                h1 = actsp.tile([128, nb * 6], BF16, tag="h1")
                h1v = h1[:].rearrange("p (r f) -> p r f", f=6)
                nc.vector.memset(h1v[:, :, 5:6], 1.0)
                nc.vector.tensor_mul(
                    h1[:].rearrange("p (c d f) -> p c d f", d=L0C, f=6)[:, :, :, 0:5],
                    sio0[:].rearrange("p (c g d f) -> p c g d f", g=2, d=L0C, f=5)[:, :, 1],
                    tc1[:].rearrange("p (c d f) -> p c d f", d=L0C, f=5),
                )

                # ---- L1: transpose + matmul into g1
                g1_ps = g1psp.tile([128, g1b * 512], F32, tag="g1")
                h1t_ps = h1tpsp.tile([L1C * 6, n1ch * 128], BF16, tag="h1tps")
                for c in range(n1ch):
                    nc.tensor.transpose(
                        h1t_ps[:, c * 128 : (c + 1) * 128],
                        h1[:, c * L1C * 6 : (c + 1) * L1C * 6],
                        id_sb[:],
                    )
                h1t_sb = h1tsbp.tile([L1C * 6, n1ch * 128], BF16, tag="h1tsb")
                nc.vector.tensor_copy(h1t_sb[:], h1t_ps[:])
                for c in range(n1ch):
                    off = (c // L1_PER_BANK) * 512 + (c % L1_PER_BANK) * 240
                    nc.tensor.matmul(
                        g1_ps[:, off : off + 240],
                        h1t_sb[:, c * 128 : (c + 1) * 128],
                        w1_sb[:],
                        start=True,
                        stop=True,
                    )

                # ---- L1 activations
                g1v = (
                    g1_ps[:]
                    .rearrange("p (b x) -> p b x", x=512)[:, :, : L1_PER_BANK * 240]
                    .rearrange("p b (c n) -> p b c n", n=240)
                )
                sio1 = actsp.tile([128, n1ch * 160], BF16, tag="sio1")
                sio1v = sio1[:].rearrange("p (b c n) -> p b c n", n=160, c=L1_PER_BANK)
                nc.scalar.activation(sio1v, g1v[:, :, :, 0:160], AF.Sigmoid)
                tg1 = actsp.tile([128, n1ch * 80], BF16, tag="tg1")
                tg1v = tg1[:].rearrange("p (b c n) -> p b c n", n=80, c=L1_PER_BANK)
                nc.scalar.activation(tg1v, g1v[:, :, :, 160:240], AF.Tanh)

                c2 = actsp.tile([128, nb * HID], BF16, tag="c2")
                nc.vector.tensor_mul(
                    c2[:].rearrange("p (c n) -> p c n", n=80),
                    sio1[:].rearrange("p (c n) -> p c n", n=160)[:, :, 0:80],
                    tg1[:].rearrange("p (c n) -> p c n", n=80),
                )
                tc2 = actsp.tile([128, nb * HID], BF16, tag="tc2")
                nc.scalar.activation(tc2[:], c2[:], AF.Tanh)
                v = actsp.tile([128, nb * HID], BF16, tag="v")
                nc.vector.tensor_mul(
                    v[:].rearrange("p (c n) -> p c n", n=80),
                    sio1[:].rearrange("p (c n) -> p c n", n=160)[:, :, 80:160],
                    tc2[:].rearrange("p (c n) -> p c n", n=80),
                )
                t = actsp.tile([128, nb * HID], BF16, tag="t")
                nc.vector.tensor_mul(t[:], v[:], wr_sb[:, : nb * HID])

                # ---- final reduce + tanh(+bias) + store
                ypre = actsp.tile([128, nb], F32, tag="ypre")
                nc.vector.tensor_reduce(
                    ypre[:].rearrange("p (r o) -> p r o", o=1),
                    t[:].rearrange("p (r f) -> p r f", f=HID),
                    mybir.AxisListType.X,
                    mybir.AluOpType.add,
                )
                y_tile = youtp.tile([128, nb], F32, tag="y")
                nc.scalar.activation(y_tile[:], ypre[:], AF.Tanh, bias=bl_sb[:, 0:1])
                nc.sync.dma_start(
                    out=y_d[s0 : s0 + S, 0:1].rearrange("(p r) o -> p (r o)", p=128),
                    in_=y_tile[:],
                )

                s0 += S

    nc.compile()
    return nc


def _build_inputs(x, W_ih0, W_hh0, b_ih0, b_hh0, W_ih1, W_hh1, b_ih1, b_hh1, W_lin, b_lin):
    bf16 = ml_dtypes.bfloat16
    b0 = (np.asarray(b_ih0) + np.asarray(b_hh0)).astype(np.float32)
    b1 = (np.asarray(b_ih1) + np.asarray(b_hh1)).astype(np.float32)
    W0 = np.asarray(W_ih0, np.float32)
    W1 = np.asarray(W_ih1, np.float32)
    sel = {"i": range(0, 5), "g": range(10, 15), "o": range(15, 20)}

    def blockdiag(W, b, chunk, krows):
        # krows per block = W.shape[1] (+1 bias row); col layout [i | o | g]
        kin = W.shape[1]
        out = np.zeros((chunk * (kin + 1), chunk * 15), np.float32)
        for dr in range(chunk):
            for grp, key in enumerate(("i", "o", "g")):
                for kk, gr in enumerate(sel[key]):
                    col = grp * (chunk * 5) + dr * 5 + kk
                    out[dr * (kin + 1) : dr * (kin + 1) + kin, col] = W[gr, :]
                    out[dr * (kin + 1) + kin, col] = b[gr]
        return out.astype(bf16)

    w0blk = blockdiag(W0, b0, L0C, 22)
    w1blk = blockdiag(W1, b1, L1C, 6)
    wrep = np.tile(np.asarray(W_lin, np.float32)[0], 96 * 128).reshape(128, 96 * HID).astype(bf16)
    blin = np.full((128, 1), float(np.asarray(b_lin)[0]), np.float32)
    ident = np.eye(128, dtype=bf16)

    x = np.asarray(x, np.float32)
    x22 = np.empty((B, 22), np.float32)
    x22[:, :21] = x
    x22[:, 21] = 1.0

    in_maps = []
    for c in range(NCORES):
        in_maps.append(
            {
                "x22": x22[c * BC : (c + 1) * BC],
                "w0blk": w0blk,
                "w1blk": w1blk,
                "wrep": wrep,
                "blin": blin,
                "ident": ident,
            }
        )
    return in_maps


def _reference_numpy(x, h0, c0, W_ih0, W_hh0, b_ih0, b_hh0, W_ih1, W_hh1, b_ih1, b_hh1, W_lin, b_lin):
    # general fallback (never taken for the spec'd zero-state inputs)
    def cell(x_, h, c, Wi, Wh, bi, bh):
        g = x_ @ Wi.T + h @ Wh.T + (bi + bh)
        i, f, gg, o = np.split(g, 4, axis=-1)
        sig = lambda z: 1.0 / (1.0 + np.exp(-z))
        cn = sig(f) * c + sig(i) * np.tanh(gg)
        return sig(o) * np.tanh(cn), cn

    h1, _ = cell(x, h0[0], c0[0], W_ih0, W_hh0, b_ih0, b_hh0)
    h2, _ = cell(h1, h0[1], c0[1], W_ih1, W_hh1, b_ih1, b_hh1)
    return np.tanh(h2 @ W_lin.T + b_lin).astype(np.float32)


def kernel(x, h0, c0, W_ih0, W_hh0, b_ih0, b_hh0, W_ih1, W_hh1, b_ih1, b_hh1, W_lin, b_lin):
    global LAST_RESULTS
    args = dict(
        x=np.asarray(x), h0=np.asarray(h0), c0=np.asarray(c0),
        W_ih0=np.asarray(W_ih0), W_hh0=np.asarray(W_hh0),
        b_ih0=np.asarray(b_ih0), b_hh0=np.asarray(b_hh0),
        W_ih1=np.asarray(W_ih1), W_hh1=np.asarray(W_hh1),
        b_ih1=np.asarray(b_ih1), b_hh1=np.asarray(b_hh1),
        W_lin=np.asarray(W_lin), b_lin=np.asarray(b_lin),
    )
    if np.any(args["h0"]) or np.any(args["c0"]):
        return _reference_numpy(**args)

    from concourse.bass_utils import run_bass_kernel_spmd

    if "nc" not in _CACHE:
        _CACHE["nc"] = _build_program()
    nc = _CACHE["nc"]

    in_maps = _build_inputs(
        args["x"], args["W_ih0"], args["W_hh0"], args["b_ih0"], args["b_hh0"],
        args["W_ih1"], args["W_hh1"], args["b_ih1"], args["b_hh1"],
        args["W_lin"], args["b_lin"],
    )
    trace = bool(int(os.environ.get("TRN_TRACE", "0")))
    res = run_bass_kernel_spmd(nc, in_maps, list(range(NCORES)), trace=trace)
    LAST_RESULTS = res
    return np.concatenate([res.results[i]["y"] for i in range(NCORES)], axis=0)
